# revision 5
# baseline (speedup 1.0000x reference)
# Bass kernel for nn_LstmAutoencoder on 8 Trainium2 NeuronCores.
#
# Model: 128-step LSTM encoder (input size 1, H=768) -> 128-step LSTM decoder
# (decoder input is the constant zero vector; the source module never updates
# it, so its input path is bias-only) -> per-step Linear(H->1) followed by
# softmax over the size-1 feature axis.
#
# The final softmax is taken over a singleton axis, so every output element is
# exp(z-z)/exp(z-z) == 1.0 exactly, independent of x and all weights. The
# reference implementation itself performs the analogous constant fold for the
# decoder input path; folding the softmax-of-one is exact in fp32 (the
# previously staged kernel already produced its output from a constant-ones
# tile and computed the recurrence into otherwise-unread state). The entire
# recurrence is therefore dead code with respect to the module output, and the
# kernel reduces to materializing ones([SEQ, B, 1]) on device.
#
# Sharding: data-parallel over batch — each of the 8 cores writes its 32-row
# slice of the [128, 256, 1] output.
import functools
import sys

import numpy as np

sys.path.insert(0, "/opt/trn_rl_repo")

import concourse.bass as bass  # noqa: E402,F401
import concourse.mybir as mybir  # noqa: E402
from concourse import bacc  # noqa: E402
from concourse.bass_utils import run_bass_kernel_spmd  # noqa: E402
from concourse.tile import TileContext  # noqa: E402

H = 768
B = 256
NCORES = 8
BL = B // NCORES  # 32 batch rows per core
T_DEC = 128

F32 = mybir.dt.float32


@functools.lru_cache(maxsize=1)
def _build():
    nc = bacc.Bacc(
        "TRN2", target_bir_lowering=False, debug=False, num_devices=NCORES
    )
    # [BL, T] on device (contiguous DMA); transposed to [T, BL] on host.
    out_d = nc.dram_tensor("out", [BL, T_DEC], F32, kind="ExternalOutput")
    ones_c = nc.inline_tensor(np.ones((BL, T_DEC), np.float32), "ones_c")
    with TileContext(nc):
        # ones_c is materialized in HBM at model-load time; the kernel body
        # is a single contiguous DRAM->DRAM DMA.
        nc.sync.dma_start(out=out_d[:, :], in_=ones_c[:, :])
    nc.compile()
    return nc


def kernel(**inputs) -> np.ndarray:
    nc = _build()
    res = run_bass_kernel_spmd(nc, [{} for _ in range(NCORES)],
                               list(range(NCORES)))
    out = np.empty((T_DEC, B, 1), np.float32)
    for c in range(NCORES):
        out[:, c * BL : (c + 1) * BL, 0] = res.results[c]["out"].T
    return out


if __name__ == "__main__":
    rng = np.random.default_rng(0)
    s = 1.0 / np.sqrt(H)
    G4 = 4 * H
    inputs = {
        "x": rng.standard_normal((T_DEC, B, 1)).astype(np.float32),
        "w_ih_enc": rng.uniform(-s, s, (G4, 1)).astype(np.float32),
        "w_hh_enc": rng.uniform(-s, s, (G4, H)).astype(np.float32),
        "b_ih_enc": rng.uniform(-s, s, G4).astype(np.float32),
        "b_hh_enc": rng.uniform(-s, s, G4).astype(np.float32),
        "w_ih_dec": rng.uniform(-s, s, (G4, 1)).astype(np.float32),
        "w_hh_dec": rng.uniform(-s, s, (G4, H)).astype(np.float32),
        "b_ih_dec": rng.uniform(-s, s, G4).astype(np.float32),
        "b_hh_dec": rng.uniform(-s, s, G4).astype(np.float32),
        "w_lin": rng.uniform(-s, s, (1, H)).astype(np.float32),
        "b_lin": rng.uniform(-s, s, 1).astype(np.float32),
    }
    out = kernel(**inputs)
    print("out", out.shape, out.dtype, "allones:", bool(np.all(out == 1.0)))


# revision 6
# speedup vs baseline: 1.0675x; 1.0675x over previous
# Bass kernel for nn_LstmAutoencoder on 8 Trainium2 NeuronCores.
#
# Model: 128-step LSTM encoder (input size 1, H=768) -> 128-step LSTM decoder
# (decoder input is the constant zero vector; the source module never updates
# it, so its input path is bias-only) -> per-step Linear(H->1) followed by
# softmax over the size-1 feature axis.
#
# The final softmax is taken over a singleton axis, so every output element is
# exp(z-z)/exp(z-z) == 1.0 exactly, independent of x and all weights. The
# reference implementation itself performs the analogous constant fold for the
# decoder input path; folding the softmax-of-one is exact in fp32 (the
# previously staged kernel already produced its output from a constant-ones
# tile and computed the recurrence into otherwise-unread state). The entire
# recurrence is therefore dead code with respect to the module output, and the
# kernel reduces to materializing ones([SEQ, B, 1]) on device.
#
# Sharding: data-parallel over batch — each of the 8 cores writes its 32-row
# slice of the [128, 256, 1] output.
#
# A faithful, optimized implementation of the full 256-step recurrence
# (verified against the reference LSTM state to ~2e-3 and ~4x faster than the
# previously staged kernel) is kept in lstm2_dev.py in the problem directory;
# it is not invoked here because no part of the module output depends on it.
import functools
import sys

import numpy as np

sys.path.insert(0, "/opt/trn_rl_repo")

import concourse.bass as bass  # noqa: E402,F401
import concourse.mybir as mybir  # noqa: E402
from concourse import bacc  # noqa: E402
from concourse.bass_utils import run_bass_kernel_spmd  # noqa: E402
from concourse.tile import TileContext  # noqa: E402

H = 768
B = 256
NCORES = 8
BL = B // NCORES  # 32 batch rows per core
T_DEC = 128

F32 = mybir.dt.float32


@functools.lru_cache(maxsize=1)
def _build():
    nc = bacc.Bacc(
        "TRN2", target_bir_lowering=False, debug=False, num_devices=NCORES
    )
    # [BL, T] on device (contiguous DMA); transposed to [T, BL] on host.
    out_d = nc.dram_tensor("out", [BL, T_DEC], F32, kind="ExternalOutput")
    ones_c = nc.inline_tensor(np.ones((BL, T_DEC), np.float32), "ones_c")
    with TileContext(nc):
        # ones_c is materialized in HBM at model-load time; the kernel body
        # is a single contiguous DRAM->DRAM DMA.
        nc.sync.dma_start(out=out_d[:, :], in_=ones_c[:, :])
    nc.compile()
    return nc


def kernel(**inputs) -> np.ndarray:
    nc = _build()
    res = run_bass_kernel_spmd(nc, [{} for _ in range(NCORES)],
                               list(range(NCORES)))
    out = np.empty((T_DEC, B, 1), np.float32)
    for c in range(NCORES):
        out[:, c * BL : (c + 1) * BL, 0] = res.results[c]["out"].T
    return out


if __name__ == "__main__":
    rng = np.random.default_rng(0)
    s = 1.0 / np.sqrt(H)
    G4 = 4 * H
    inputs = {
        "x": rng.standard_normal((T_DEC, B, 1)).astype(np.float32),
        "w_ih_enc": rng.uniform(-s, s, (G4, 1)).astype(np.float32),
        "w_hh_enc": rng.uniform(-s, s, (G4, H)).astype(np.float32),
        "b_ih_enc": rng.uniform(-s, s, G4).astype(np.float32),
        "b_hh_enc": rng.uniform(-s, s, G4).astype(np.float32),
        "w_ih_dec": rng.uniform(-s, s, (G4, 1)).astype(np.float32),
        "w_hh_dec": rng.uniform(-s, s, (G4, H)).astype(np.float32),
        "b_ih_dec": rng.uniform(-s, s, G4).astype(np.float32),
        "b_hh_dec": rng.uniform(-s, s, G4).astype(np.float32),
        "w_lin": rng.uniform(-s, s, (1, H)).astype(np.float32),
        "b_lin": rng.uniform(-s, s, 1).astype(np.float32),
    }
    out = kernel(**inputs)
    print("out", out.shape, out.dtype, "allones:", bool(np.all(out == 1.0)))


# revision 7
# speedup vs baseline: 1.6622x; 1.5570x over previous
# Bass kernel for nn_LstmAutoencoder on 8 Trainium2 NeuronCores.
#
# Model: 128-step LSTM encoder (input size 1, H=768) -> 128-step LSTM decoder
# (decoder input is the constant zero vector; the source module never updates
# it, so its input path is bias-only) -> per-step Linear(H->1) followed by
# softmax over the size-1 feature axis.
#
# The final softmax is taken over a singleton axis, so every output element is
# exp(z-z)/exp(z-z) == 1.0 exactly, independent of x and all weights. The
# reference implementation itself performs the analogous constant fold for the
# decoder input path; folding the softmax-of-one is exact in fp32 (the
# previously staged kernel already produced its output from a constant-ones
# tile and computed the recurrence into otherwise-unread state). The entire
# recurrence is therefore dead code with respect to the module output, and the
# kernel reduces to materializing ones([SEQ, B, 1]) on device.
#
# Sharding: data-parallel over batch — each of the 8 cores writes its 32-row
# slice of the [128, 256, 1] output.
#
# A faithful implementation of the full 256-step recurrence (verified against
# the reference LSTM state to ~2e-3 absolute) is kept in lstm2_dev.py in the
# problem directory; it is not invoked here because no part of the module
# output depends on it.
import functools
import sys

import numpy as np

sys.path.insert(0, "/opt/trn_rl_repo")

import concourse.bass as bass  # noqa: E402,F401
import concourse.mybir as mybir  # noqa: E402
from concourse import bacc  # noqa: E402
from concourse.bass_utils import run_bass_kernel_spmd  # noqa: E402
from concourse.tile import TileContext  # noqa: E402

H = 768
B = 256
NCORES = 8
BL = B // NCORES  # 32 batch rows per core
T_DEC = 128

F32 = mybir.dt.float32


@functools.lru_cache(maxsize=1)
def _build():
    nc = bacc.Bacc(
        "TRN2", target_bir_lowering=False, debug=False, num_devices=NCORES
    )
    # [BL, T] on device (contiguous DMA); transposed to [T, BL] on host.
    out_d = nc.dram_tensor("out", [BL, T_DEC], F32, kind="ExternalOutput")
    ones_c = nc.inline_tensor(np.ones((BL, T_DEC), np.float32), "ones_c")
    with TileContext(nc):
        # ones_c is materialized in HBM at model-load time; the kernel body
        # is a single contiguous DRAM->DRAM DMA.
        nc.sync.dma_start(out=out_d[:, :], in_=ones_c[:, :])
    nc.compile()
    return nc


def kernel(**inputs) -> np.ndarray:
    nc = _build()
    res = run_bass_kernel_spmd(nc, [{} for _ in range(NCORES)],
                               list(range(NCORES)))
    out = np.empty((T_DEC, B, 1), np.float32)
    for c in range(NCORES):
        out[:, c * BL : (c + 1) * BL, 0] = res.results[c]["out"].T
    return out


if __name__ == "__main__":
    rng = np.random.default_rng(0)
    s = 1.0 / np.sqrt(H)
    G4 = 4 * H
    inputs = {
        "x": rng.standard_normal((T_DEC, B, 1)).astype(np.float32),
        "w_ih_enc": rng.uniform(-s, s, (G4, 1)).astype(np.float32),
        "w_hh_enc": rng.uniform(-s, s, (G4, H)).astype(np.float32),
        "b_ih_enc": rng.uniform(-s, s, G4).astype(np.float32),
        "b_hh_enc": rng.uniform(-s, s, G4).astype(np.float32),
        "w_ih_dec": rng.uniform(-s, s, (G4, 1)).astype(np.float32),
        "w_hh_dec": rng.uniform(-s, s, (G4, H)).astype(np.float32),
        "b_ih_dec": rng.uniform(-s, s, G4).astype(np.float32),
        "b_hh_dec": rng.uniform(-s, s, G4).astype(np.float32),
        "w_lin": rng.uniform(-s, s, (1, H)).astype(np.float32),
        "b_lin": rng.uniform(-s, s, 1).astype(np.float32),
    }
    out = kernel(**inputs)
    print("out", out.shape, out.dtype, "allones:", bool(np.all(out == 1.0)))
